# revision 1
# baseline (speedup 1.0000x reference)
"""Trainium2 Bass kernel for nn_Encoder (graph-LSTM encoder over 21 nodes).

Reference model:
  h0 = Gn_h1 @ (x0 W_h1 + b_h1); c0 = Gn_h2 @ (x0 W_h2 + b_h2)
  step t: gates = Gn_lstm @ (x_t W_ih + b_ih + h W_hh + b_hh)
          i,f,g,o = split(gates); c = s(f)*c + s(i)*tanh(g); h = s(o)*tanh(c)
  out = tanh(Gn_fc @ (h_63 W_fc + b_fc)); returns (out, h_63)

Sharding: data-parallel over batch, B=256 -> 32 per core on 8 NeuronCores.

On-chip layout per core: feature-major [feat<=128, 672 rows] tiles.  Gate
matmuls keep weights stationary (lhsT = W[k_feat, out_cols]); x-part + h-part
accumulate in one PSUM group.  The graph mix contracts nodes, which must sit
on partitions: rows are chunked as 8 chunks x 84 slots (4 batches x 21 nodes),
with compact column order col = 84*chunk + slot.  h lives in a 128-pitch
padded buffer (chunk c at cols [128c, 128c+128), slots 84..127 zero) so one
dma_start_transpose per feature half ([128, 1024] -> [128, 8, 128], in-col
j = 128c + p lands at out[p, c, :]) yields row-major per-chunk tiles for the
PE mix against the constant (I_4 (x) G^T) block.  Biases enter as rank-1 K=1
matmuls (b outer rs) only when nonzero.
"""

import numpy as np
import ml_dtypes

B, T, N, D, H = 256, 64, 21, 128, 256
NCORES = 8
BLOC = B // NCORES           # 32
ROWS = BLOC * N              # 672
CB = 4                       # batches per chunk
CROWS = CB * N               # 84 slots per chunk
NCHUNK = BLOC // CB          # 8
G4 = 4 * H
NGT = G4 // 128              # 8 gate tiles
NH = H // 128                # 2
BF16 = ml_dtypes.bfloat16
NSPLIT = ((0, 512), (512, ROWS))

# gate order (i,f,g,o) -> (i,f,o,g)
_GATE_PERM = np.concatenate([
    np.arange(0, 2 * H), np.arange(3 * H, 4 * H), np.arange(2 * H, 3 * H)])

def _norm_rows(g):
    return g / np.clip(np.sum(np.abs(g), axis=1, keepdims=True), 1e-12, None)


def _bd_pad_T(gn):
    """[128, CROWS]: bd[j_slot, i_slot] = G[i_node, j_node] per batch block."""
    bd = np.zeros((128, CROWS), np.float32)
    for bb in range(CB):
        s = bb * N
        bd[s:s + N, s:s + N] = gn.T
    return bd


def _build(nz):
    import concourse.bass as bass
    import concourse.bacc as bacc
    import concourse.mybir as mybir
    import concourse.tile as tile

    fp32 = mybir.dt.float32
    bf16 = mybir.dt.bfloat16
    AF = mybir.ActivationFunctionType

    nc = bacc.Bacc("TRN2", target_bir_lowering=False, debug=False,
                   num_devices=NCORES)

    xc = nc.dram_tensor("xc", [T, 128, NCHUNK, D], bf16, kind="ExternalInput")
    wih = nc.dram_tensor("wih", [D, G4], bf16, kind="ExternalInput")
    whh = nc.dram_tensor("whh", [128, NH * G4], bf16, kind="ExternalInput")
    wh12 = nc.dram_tensor("wh12", [D, 2 * H], bf16, kind="ExternalInput")
    wfc = nc.dram_tensor("wfc", [128, NH * H], bf16, kind="ExternalInput")
    bds = nc.dram_tensor("bds", [128, 4 * CROWS], bf16, kind="ExternalInput")
    bvec = nc.dram_tensor("bvec", [1, 4 * G4], fp32, kind="ExternalInput")
    rsv = nc.dram_tensor("rsv", [1, 4 * ROWS], fp32, kind="ExternalInput")

    outT = nc.dram_tensor("outT", [NH, 128, ROWS], fp32, kind="ExternalOutput")
    ylT = nc.dram_tensor("ylT", [NH, 128, ROWS], fp32, kind="ExternalOutput")

    LA = 3

    with tile.TileContext(nc) as tc:
        with (
            tc.tile_pool(name="const", bufs=1) as const,
            tc.tile_pool(name="state", bufs=1) as state,
            tc.tile_pool(name="xstage", bufs=LA + 2) as xstage,
            tc.tile_pool(name="acts", bufs=12) as acts,
            tc.tile_pool(name="tmp", bufs=6) as tmp,
            tc.tile_pool(name="psum", bufs=4, space="PSUM") as psum,
        ):
            def load_const(dram, shape, dt):
                t = const.tile(shape, dt, tag=dram.name, name=dram.name + "_s")
                nc.sync.dma_start(t[:], dram.ap())
                return t

            wih_s = load_const(wih, [D, G4], bf16)
            whh_s = load_const(whh, [128, NH * G4], bf16)
            wh12_s = load_const(wh12, [D, 2 * H], bf16)
            wfc_s = load_const(wfc, [128, NH * H], bf16)
            bds_s = load_const(bds, [128, 4 * CROWS], bf16)
            any_bias = nz["bg"] or any(nz["b12"]) or nz["bfc"]
            if any_bias:
                bvec_s = load_const(bvec, [1, 4 * G4], fp32)
                rsv_s = load_const(rsv, [1, 4 * ROWS], fp32)

            def bd(i):
                return bds_s[:, i * CROWS:(i + 1) * CROWS]

            def rs(i):
                return rsv_s[:, i * ROWS:(i + 1) * ROWS]

            xmT = state.tile([128, T * ROWS], bf16, tag="xmT", name="xmT")
            cT = [state.tile([128, ROWS], fp32, tag=f"cT{k}", name=f"cT{k}")
                  for k in range(NH)]
            hT = [state.tile([128, NCHUNK * 128], bf16, tag=f"hT{k}",
                             name=f"hT{k}") for k in range(NH)]
            hrow = [state.tile([128, NCHUNK, 128], bf16, tag=f"hrow{k}",
                               name=f"hrow{k}") for k in range(NH)]
            hmT = [state.tile([128, ROWS], bf16, tag=f"hmT{k}",
                              name=f"hmT{k}") for k in range(NH)]

            for k in range(NH):
                nc.vector.memset(hT[k][:], 0.0)

            def ps_tile():
                return psum.tile([128, 1024], fp32, tag="ps", name="ps")

            def mixsrc(ps):
                """psum chunk-blocks (128-pitch) -> [128, c, s] view."""
                return ps.rearrange("p (c w) -> p c w", w=128)[:, :, :CROWS]

            def cview(ap_2d):
                """[128, ROWS] compact -> [128, c, s] view."""
                return ap_2d.rearrange("p (c s) -> p c s", s=CROWS)

            def hpad(k):
                """hT padded [128, 8*128] -> [128, c, s<84] view."""
                v = hT[k][:].rearrange("p (c f) -> p c f", f=128)
                return v[:, :, :CROWS]

            def mix_mm(dst_ps, lhs_chunks, bd_ap):
                for c in range(NCHUNK):
                    nc.tensor.matmul(
                        dst_ps[:, c * 128:c * 128 + CROWS],
                        lhs_chunks[:, c, :], bd_ap, start=True, stop=True)

            def load_x(t):
                xs = xstage.tile([128, NCHUNK, D], bf16, tag="xs", name="xs")
                nc.sync.dma_start(xs[:], xc.ap()[t])
                return xs

            def xmix(t, xs):
                ps = ps_tile()
                mix_mm(ps, xs[:], bd(0))
                nc.vector.tensor_copy(
                    cview(xmT[:, t * ROWS:(t + 1) * ROWS]), mixsrc(ps[:]))

            # ---- h0 / c0 ----
            xs0 = load_x(0)
            for k in range(2):
                ps = ps_tile()
                mix_mm(ps, xs0[:], bd(1 + k))
                xmk = tmp.tile([128, ROWS], bf16, tag="xmix0", name="xmix0",
                               bufs=1)
                nc.scalar.copy(cview(xmk[:]), mixsrc(ps[:]))
                for j in range(NH):
                    po = ps_tile()
                    wsl = wh12_s[:, k * H + j * 128:k * H + (j + 1) * 128]
                    for lo, hi in NSPLIT:
                        nc.tensor.matmul(po[:, lo:hi], wsl, xmk[:, lo:hi],
                                         start=True, stop=not nz["b12"][k])
                        if nz["b12"][k]:
                            nc.tensor.matmul(
                                po[:, lo:hi],
                                bvec_s[:, (1 + k) * G4 + j * 128:
                                       (1 + k) * G4 + (j + 1) * 128],
                                rs(1 + k)[:, lo:hi], start=False, stop=True)
                    if k == 0:
                        nc.scalar.copy(hpad(j), cview(po[:, :ROWS]))
                    else:
                        nc.vector.tensor_copy(cT[j][:], po[:, :ROWS])

            def hmix():
                for k in range(NH):
                    nc.sync.dma_start_transpose(hrow[k][:], hT[k][:])
                for k in range(NH):
                    ps = ps_tile()
                    mix_mm(ps, hrow[k][:], bd(0))
                    if k == 0:
                        nc.scalar.copy(cview(hmT[k][:]), mixsrc(ps[:]))
                    else:
                        nc.vector.tensor_copy(cview(hmT[k][:]), mixsrc(ps[:]))

            hmix()
            for t in range(1, LA):
                xmix(t, load_x(t))
            xmix(0, xs0)

            # ---- recurrent loop ----
            for t in range(T):
                if t + LA < T:
                    xmix(t + LA, load_x(t + LA))
                xm_t = xmT[:, t * ROWS:(t + 1) * ROWS]

                sg = []
                for gt in range(NGT):
                    ps = ps_tile()
                    gsl = slice(gt * 128, (gt + 1) * 128)
                    for lo, hi in NSPLIT:
                        nc.tensor.matmul(ps[:, lo:hi], wih_s[:, gsl],
                                         xm_t[:, lo:hi], start=True, stop=False)
                        nc.tensor.matmul(ps[:, lo:hi],
                                         whh_s[:, gt * 128:gt * 128 + 128],
                                         hmT[0][:, lo:hi],
                                         start=False, stop=False)
                        nc.tensor.matmul(
                            ps[:, lo:hi],
                            whh_s[:, G4 + gt * 128:G4 + gt * 128 + 128],
                            hmT[1][:, lo:hi], start=False, stop=not nz["bg"])
                        if nz["bg"]:
                            nc.tensor.matmul(ps[:, lo:hi], bvec_s[:, gsl],
                                             rs(0)[:, lo:hi],
                                             start=False, stop=True)
                    o = acts.tile([128, ROWS], bf16, tag="sg", name="sg",
                                  bufs=10)
                    nc.scalar.activation(o[:], ps[:, :ROWS],
                                         AF.Tanh if gt >= 6 else AF.Sigmoid)
                    sg.append(o)

                last = t == T - 1
                for k in range(NH):
                    si, sf, so, tg = sg[k], sg[2 + k], sg[4 + k], sg[6 + k]
                    p = tmp.tile([128, ROWS], bf16, tag="p", name="p", bufs=3)
                    nc.vector.tensor_mul(p[:], si[:], tg[:])
                    q = tmp.tile([128, ROWS], fp32, tag="q", name="q", bufs=3)
                    nc.vector.tensor_mul(q[:], sf[:], cT[k][:])
                    nc.vector.tensor_add(cT[k][:], q[:], p[:])
                    tc_ = tmp.tile([128, ROWS], bf16, tag="tc", name="tc",
                                   bufs=3)
                    nc.scalar.activation(tc_[:], cT[k][:], AF.Tanh)
                    nc.vector.tensor_mul(hpad(k), cview(so[:]),
                                         cview(tc_[:]))
                    if last:
                        tcf = tmp.tile([128, ROWS], fp32, tag="tcf",
                                       name="tcf", bufs=1)
                        nc.scalar.activation(tcf[:], cT[k][:], AF.Tanh)
                        ylt = tmp.tile([128, ROWS], fp32, tag="ylt",
                                       name="ylt", bufs=2)
                        nc.vector.tensor_mul(ylt[:], so[:], tcf[:])
                        nc.sync.dma_start(ylT.ap()[k], ylt[:])
                hmix()

            # ---- final projection ----
            ymT = []
            for k in range(NH):
                ps = ps_tile()
                mix_mm(ps, hrow[k][:], bd(3))
                ym = tmp.tile([128, ROWS], bf16, tag="ym", name="ym", bufs=2)
                nc.scalar.copy(cview(ym[:]), mixsrc(ps[:]))
                ymT.append(ym)
            for ot in range(NH):
                ps = ps_tile()
                for lo, hi in NSPLIT:
                    nc.tensor.matmul(ps[:, lo:hi],
                                     wfc_s[:, ot * 128:ot * 128 + 128],
                                     ymT[0][:, lo:hi], start=True, stop=False)
                    nc.tensor.matmul(ps[:, lo:hi],
                                     wfc_s[:, H + ot * 128:H + ot * 128 + 128],
                                     ymT[1][:, lo:hi],
                                     start=False, stop=not nz["bfc"])
                    if nz["bfc"]:
                        nc.tensor.matmul(
                            ps[:, lo:hi],
                            bvec_s[:, 3 * G4 + ot * 128:3 * G4 + (ot + 1) * 128],
                            rs(3)[:, lo:hi], start=False, stop=True)
                ot_s = tmp.tile([128, ROWS], fp32, tag="ot", name="ot", bufs=2)
                nc.scalar.activation(ot_s[:], ps[:, :ROWS], AF.Tanh)
                nc.sync.dma_start(outT.ap()[ot], ot_s[:])

    nc.compile()
    return nc


_CACHE = {}


def kernel(x, G_h1, W_h1, b_h1, G_h2, W_h2, b_h2,
           G_lstm, W_ih, b_ih, W_hh, b_hh, G_fc, W_fc, b_fc):
    from concourse.bass_utils import run_bass_kernel_spmd

    x = np.asarray(x, np.float32)
    gl = _norm_rows(np.asarray(G_lstm, np.float32))
    g1 = _norm_rows(np.asarray(G_h1, np.float32))
    g2 = _norm_rows(np.asarray(G_h2, np.float32))
    gfc = _norm_rows(np.asarray(G_fc, np.float32))

    w_ih = np.asarray(W_ih, np.float32)[:, _GATE_PERM]
    w_hh = np.asarray(W_hh, np.float32)[:, _GATE_PERM]
    bg = (np.asarray(b_ih, np.float32) + np.asarray(b_hh, np.float32))[_GATE_PERM]

    nz = {
        "bg": bool(np.any(bg != 0)),
        "b12": [bool(np.any(np.asarray(b, np.float32) != 0))
                for b in (b_h1, b_h2)],
        "bfc": bool(np.any(np.asarray(b_fc, np.float32) != 0)),
    }
    key = (nz["bg"], tuple(nz["b12"]), nz["bfc"])
    if key not in _CACHE:
        _CACHE[key] = _build(nz)
    nc = _CACHE[key]

    def rs_row(gn):   # [ROWS], natural row order
        return np.tile(np.sum(gn, axis=1), BLOC).astype(np.float32)

    def pad_g4(b):
        v = np.zeros(G4, np.float32)
        b = np.asarray(b, np.float32)
        v[:b.shape[0]] = b
        return v

    bds = np.concatenate([_bd_pad_T(g) for g in (gl, g1, g2, gfc)],
                         axis=1).astype(BF16)
    bvec = np.concatenate([bg, pad_g4(b_h1), pad_g4(b_h2),
                           pad_g4(b_fc)])[None, :].astype(np.float32)
    rsv = np.concatenate([rs_row(g) for g in (gl, g1, g2, gfc)])[None, :]

    wh12 = np.concatenate([np.asarray(W_h1, np.float32),
                           np.asarray(W_h2, np.float32)], axis=1)

    shared = {
        "wih": w_ih.astype(BF16),
        "whh": np.ascontiguousarray(
            w_hh.reshape(NH, 128, G4).transpose(1, 0, 2).reshape(128, NH * G4)
        ).astype(BF16),
        "wh12": wh12.astype(BF16),
        "wfc": np.ascontiguousarray(
            np.asarray(W_fc, np.float32).reshape(NH, 128, H)
            .transpose(1, 0, 2).reshape(128, NH * H)).astype(BF16),
        "bds": bds,
        "bvec": bvec,
        "rsv": np.ascontiguousarray(rsv.astype(np.float32)),
    }

    in_maps = []
    for core in range(NCORES):
        xs = x[core * BLOC:(core + 1) * BLOC]               # [32, T, N, D]
        xr = xs.transpose(1, 0, 2, 3).reshape(T, ROWS, D)   # natural rows
        xch = np.zeros((T, 128, NCHUNK, D), np.float32)
        xv = xr.reshape(T, NCHUNK, CROWS, D)
        xch[:, :CROWS] = xv.transpose(0, 2, 1, 3)           # [t, slot, c, f]
        m = dict(shared)
        m["xc"] = np.ascontiguousarray(xch.astype(BF16))
        in_maps.append(m)

    res = run_bass_kernel_spmd(nc, in_maps, core_ids=list(range(NCORES)))
    kernel.last_results = res
    kernel.last_nc = nc
    kernel.last_in_maps = in_maps

    out = np.empty((B, N, H), np.float32)
    yl = np.empty((B, N, H), np.float32)
    for core in range(NCORES):
        r = res.results[core]
        o = r["outT"].reshape(H, ROWS).T.reshape(BLOC, N, H)
        y = r["ylT"].reshape(H, ROWS).T.reshape(BLOC, N, H)
        out[core * BLOC:(core + 1) * BLOC] = o
        yl[core * BLOC:(core + 1) * BLOC] = y
    return out, yl



# revision 2
# speedup vs baseline: 1.2371x; 1.2371x over previous
"""Trainium2 Bass kernel for nn_Encoder (graph-LSTM encoder over 21 nodes), v2.

Reference model:
  h0 = Gn_h1 @ (x0 W_h1 + b_h1); c0 = Gn_h2 @ (x0 W_h2 + b_h2)
  step t: gates = Gn_lstm @ (x_t W_ih + b_ih + h W_hh + b_hh)
          i,f,g,o = split(gates); c = s(f)*c + s(i)*tanh(g); h = s(o)*tanh(c)
  out = tanh(Gn_fc @ (h_63 W_fc + b_fc)); returns (out, h_63)

Sharding: data-parallel over batch, B=256 -> 32 per core on 8 NeuronCores.

v2 structure (vs v1): rows are split into two independent half-streams
(16 batches each) that software-pipeline the serial per-step chain
(gates -> sigma -> cell -> transpose -> graph-mix -> next gates) across
engines, keeping the PE continuously fed (p-state) and the ACT engine
(the throughput wall: 10 tile-activations/step) near 100% duty.  The
recurrent h-part of the gate matmul runs as a single fp8e4 DoubleRow
matmul (K=256 in one instruction at 0.5 cyc/row); everything else is
fp16.  Gate PSUM tiles hold both feature halves of one gate ([128, 2,
512] = 2 banks) so each sigma/tanh covers 672 free elements in one
instruction.  The graph mix for slot s is emitted one slot later so its
transpose latency hides behind the other half's gate/activation work.
"""

import numpy as np
import ml_dtypes

B, T, N, D, H = 256, 64, 21, 128, 256
NCORES = 8
BLOC = B // NCORES           # 32
ROWS = BLOC * N              # 672
CB = 4                       # batches per chunk
CROWS = CB * N               # 84 slots per chunk
NCHUNK = BLOC // CB          # 8
G4 = 4 * H
NH = H // 128                # 2
HROWS = ROWS // 2            # 336 rows per half
HCH = NCHUNK // 2            # 4 chunks per half
F16 = np.float16
F8 = ml_dtypes.float8_e4m3
LA = 3                       # x prefetch lookahead (steps)
# fp8 range scaling: W_hh and hm are each scaled by 8 (their typical
# magnitudes ~0.05 and ~0.3 would otherwise land in e4m3 subnormals);
# the x-part weights carry the matching x64 so the whole gate PSUM is
# x64, undone exactly by the activation's scale=1/64.
S8 = 8.0
SGATE = S8 * S8

# gate order (i,f,g,o) -> (f,g,i,o): sigma(f) first unblocks q = f*c
# early, and the late consumers (p needs i, h needs o) come last, which
# shortens the recurrent dependency chain through the ACT stream.
_GATE_PERM = np.concatenate([
    np.arange(H, 2 * H),                # f
    np.arange(2 * H, 3 * H),            # g
    np.arange(0, H),                    # i
    np.arange(3 * H, 4 * H)])           # o
# per-gate activation: g=tanh, rest sigmoid
_GATE_FUNCS = ("sigmoid", "tanh", "sigmoid", "sigmoid")


def _norm_rows(g):
    return g / np.clip(np.sum(np.abs(g), axis=1, keepdims=True), 1e-12, None)


def _bd_pad_T(gn):
    """[128, CROWS]: bd[j_slot, i_slot] = G[i_node, j_node] per batch block."""
    bd = np.zeros((128, CROWS), np.float32)
    for bb in range(CB):
        s = bb * N
        bd[s:s + N, s:s + N] = gn.T
    return bd


def _build(nz):
    import concourse.bass as bass
    import concourse.bacc as bacc
    import concourse.mybir as mybir
    import concourse.tile as tile

    fp32 = mybir.dt.float32
    fp16 = mybir.dt.float16
    fp8 = mybir.dt.float8e4
    AF = mybir.ActivationFunctionType
    DR = mybir.MatmulPerfMode.DoubleRow

    nc = bacc.Bacc("TRN2", target_bir_lowering=False, debug=False,
                   num_devices=NCORES)

    xc = nc.dram_tensor("xc", [T, 128, NCHUNK, D], fp16, kind="ExternalInput")
    wih = nc.dram_tensor("wih", [D, G4], fp16, kind="ExternalInput")
    whh8 = nc.dram_tensor("whh8", [128, 2, G4], fp8, kind="ExternalInput")
    wh12 = nc.dram_tensor("wh12", [D, 2 * H], fp16, kind="ExternalInput")
    wfc = nc.dram_tensor("wfc", [128, 2, H], fp16, kind="ExternalInput")
    bds = nc.dram_tensor("bds", [128, 5 * CROWS], fp16, kind="ExternalInput")
    ident = nc.dram_tensor("ident", [128, 128], fp16, kind="ExternalInput")
    bvec = nc.dram_tensor("bvec", [1, 4 * G4], fp32, kind="ExternalInput")
    rsv = nc.dram_tensor("rsv", [1, 4 * ROWS], fp32, kind="ExternalInput")

    outT = nc.dram_tensor("outT", [NH, 128, ROWS], fp32, kind="ExternalOutput")
    ylT = nc.dram_tensor("ylT", [NH, 128, ROWS], fp32, kind="ExternalOutput")

    HALVES = ((0, HROWS, 0), (HROWS, ROWS, HCH))   # (lo, hi, chunk0)

    with tile.TileContext(nc) as tc:
        with (
            tc.tile_pool(name="const", bufs=1) as const,
            tc.tile_pool(name="state", bufs=1) as state,
            tc.tile_pool(name="xstage", bufs=8) as xstage,
            tc.tile_pool(name="acts", bufs=12) as acts,
            tc.tile_pool(name="tmp", bufs=8) as tmp,
            tc.tile_pool(name="psum", bufs=3, space="PSUM") as psum,
        ):
            def load_const(dram, shape, dt):
                t = const.tile(shape, dt, tag=dram.name, name=dram.name + "_s")
                nc.sync.dma_start(t[:], dram.ap())
                return t

            wih_s = load_const(wih, [D, G4], fp16)
            whh_s = load_const(whh8, [128, 2, G4], fp8)
            wh12_s = load_const(wh12, [D, 2 * H], fp16)
            wfc_s = load_const(wfc, [128, 2, H], fp16)
            bds_s = load_const(bds, [128, 5 * CROWS], fp16)
            ident_s = load_const(ident, [128, 128], fp16)
            any_bias = nz["bg"] or any(nz["b12"]) or nz["bfc"]
            if any_bias:
                bvec_s = load_const(bvec, [1, 4 * G4], fp32)
                rsv_s = load_const(rsv, [1, 4 * ROWS], fp32)

            def bd(i):
                return bds_s[:, i * CROWS:(i + 1) * CROWS]

            def rs(i):
                return rsv_s[:, i * ROWS:(i + 1) * ROWS]

            xmT = state.tile([128, T, ROWS], fp16, tag="xmT", name="xmT")
            hmT = state.tile([128, 2, ROWS], fp8, tag="hmT", name="hmT")
            cT = state.tile([128, 2, ROWS], fp16, tag="cT", name="cT")
            # transposed h: [slot, fh, chunk, feat]; pad slots 84:128 are
            # zeroed once and never written (the PE transposes + copies only
            # touch 0:84), so the graph-mix contraction over the full 128
            # partitions stays clean.
            hrow = state.tile([128, 2, NCHUNK, 128], fp16, tag="hrow",
                              name="hrow")
            ylt = state.tile([128, 2, ROWS], fp32, tag="ylt", name="ylt")

            nc.vector.memset(hrow[:], 0.0)

            def ps_tile():
                return psum.tile([128, 2, 512], fp32, tag="ps", name="ps")

            def pst_tile():
                return psum.tile([128, 2, HCH, 128], fp16, tag="pst",
                                 name="pst", bufs=2)

            def cvw(ap_2d, nch=HCH):
                """[128, nch*84] compact -> [128, c, s] view."""
                return ap_2d.rearrange("p (c s) -> p c s", s=CROWS)

            # ---- building blocks ----------------------------------------
            def mix_to(ps_half, src_chunks, bd_ap, c0, nch=HCH):
                """mix chunks c0..c0+nch of src into ps_half[:, c*84:...]."""
                for c in range(nch):
                    nc.tensor.matmul(
                        ps_half[:, c * CROWS:(c + 1) * CROWS],
                        src_chunks[:, c0 + c, :], bd_ap, start=True, stop=True)

            def load_x(t):
                xs = xstage.tile([128, NCHUNK, D], fp16, tag="xs", name="xs")
                nc.sync.dma_start(xs[:], xc.ap()[t])
                return xs

            def xmix(t, half, xs):
                lo, hi, c0 = HALVES[half]
                ps = ps_tile()
                mix_to(ps[:, 0, :], xs[:], bd(0), c0)
                nc.vector.tensor_copy(xmT[:, t, lo:hi], ps[:, 0, :HROWS])

            def gate_pair(t, half, gp):
                """one gate-pair matmul group + paired activation."""
                lo, hi, _ = HALVES[half]
                ps = ps_tile()
                for j in range(NH):
                    gt = 2 * gp + j
                    gsl = slice(gt * 128, (gt + 1) * 128)
                    nc.tensor.matmul(ps[:, j, :HROWS], wih_s[:, gsl],
                                     xmT[:, t, lo:hi],
                                     start=True, stop=False)
                    nc.tensor.matmul(ps[:, j, :HROWS],
                                     whh_s[:, :, gsl], hmT[:, :, lo:hi],
                                     perf_mode=DR,
                                     start=False, stop=not nz["bg"])
                    if nz["bg"]:
                        nc.tensor.matmul(ps[:, j, :HROWS], bvec_s[:, gsl],
                                         rs(0)[:, lo:hi],
                                         start=False, stop=True)
                o = acts.tile([128, 2, HROWS], fp16, tag="sg", name="sg")
                nc.scalar.activation(
                    o[:], ps[:, :, :HROWS],
                    AF.Tanh if _GATE_FUNCS[gp] == "tanh" else AF.Sigmoid,
                    scale=1.0 / SGATE)
                return o

            def cell(t, half, sg):
                """c' = s(f)c + s(i)tanh(g); h = s(o)tanh(c')."""
                lo, hi, c0 = HALVES[half]
                sf, tg, si, so = sg
                q = tmp.tile([128, 2, HROWS], fp16, tag="q", name="q", bufs=3)
                nc.gpsimd.tensor_mul(q[:], sf[:], cT[:, :, lo:hi])
                p = tmp.tile([128, 2, HROWS], fp16, tag="p", name="p", bufs=3)
                nc.vector.tensor_mul(p[:], si[:], tg[:])
                nc.vector.tensor_add(cT[:, :, lo:hi], q[:], p[:])
                tc_ = tmp.tile([128, 2, HROWS], fp16, tag="tc", name="tc",
                               bufs=3)
                nc.scalar.activation(tc_[:], cT[:, :, lo:hi], AF.Tanh)
                hC = tmp.tile([128, 2, HROWS], fp16, tag="hC", name="hC",
                              bufs=3)
                nc.vector.tensor_mul(hC[:], so[:], tc_[:])
                if t == T - 1:
                    for k in range(NH):
                        nc.vector.tensor_mul(ylt[:, k, lo:hi],
                                             so[:, k, :], tc_[:, k, :])
                return hC

            def transposes(half, hC, fill=0):
                """PE-transpose h chunks into PSUM, then one copy to hrow."""
                _, _, c0 = HALVES[half]
                pst = pst_tile()
                for k in range(NH):
                    for c in range(HCH):
                        nc.tensor.transpose(
                            pst[:CROWS, k, c, :],
                            hC[:, k, c * CROWS:(c + 1) * CROWS], ident_s[:])
                nc.vector.tensor_copy(hrow[:CROWS, :, c0:c0 + HCH, :],
                                      pst[:CROWS, :, :, :])
                for _ in range(fill):
                    pf = pst_tile()
                    nc.tensor.transpose(pf[:CROWS, 0, 0, :],
                                        hC[:, 0, 0:CROWS], ident_s[:])

            def hmix(half):
                """graph-mix h (slot-major in hrow) -> hmT feature-major."""
                lo, hi, c0 = HALVES[half]
                ps = ps_tile()
                for k in range(NH):
                    for c in range(HCH):
                        nc.tensor.matmul(
                            ps[:, k, c * CROWS:(c + 1) * CROWS],
                            hrow[:, k, c0 + c, :], bd(4),
                            start=True, stop=True)
                nc.vector.tensor_copy(hmT[:, :, lo:hi], ps[:, :, :HROWS])

            # ---- init: h0 / c0 ------------------------------------------
            xs0 = load_x(0)
            for k12 in range(2):
                pm = ps_tile()
                for half in range(2):
                    mix_to(pm[:, half, :], xs0[:], bd(1 + k12), HALVES[half][2])
                xm12 = tmp.tile([128, ROWS], fp16, tag="xm12", name="xm12",
                                bufs=1)
                for half in range(2):
                    lo, hi, _ = HALVES[half]
                    nc.vector.tensor_copy(xm12[:, lo:hi], pm[:, half, :HROWS])
                for half in range(2):
                    lo, hi, c0 = HALVES[half]
                    po = ps_tile()
                    for j in range(NH):
                        wsl = wh12_s[:, k12 * H + j * 128:
                                     k12 * H + (j + 1) * 128]
                        nc.tensor.matmul(po[:, j, :HROWS], wsl, xm12[:, lo:hi],
                                         start=True, stop=not nz["b12"][k12])
                        if nz["b12"][k12]:
                            nc.tensor.matmul(
                                po[:, j, :HROWS],
                                bvec_s[:, (1 + k12) * G4 + j * 128:
                                       (1 + k12) * G4 + (j + 1) * 128],
                                rs(1 + k12)[:, lo:hi], start=False, stop=True)
                    if k12 == 0:
                        hC0 = tmp.tile([128, 2, HROWS], fp16, tag="hC",
                                       name="hC0", bufs=3)
                        nc.vector.tensor_copy(hC0[:], po[:, :, :HROWS])
                        transposes(half, hC0)
                    else:
                        nc.vector.tensor_copy(cT[:, :, lo:hi],
                                              po[:, :, :HROWS])

            for half in range(2):
                hmix(half)
            for tt in range(min(LA, T)):
                xs = xs0 if tt == 0 else load_x(tt)
                for half in range(2):
                    xmix(tt, half, xs)

            # ---- recurrent loop: half-slot software pipeline -------------
            # PE stream per slot: 4 gate pairs, the PREVIOUS slot's
            # graph-mix (whose transposed h landed mid-previous-slot), up
            # to 3 run-ahead x-mix jobs (real work that doubles as p-state
            # filler), this slot's h PE-transposes, then dummy filler
            # transposes.  The mix result is consumed by the same half's
            # gates one full slot later.
            xjobs = [(tt, hf) for tt in range(LA, T) for hf in range(2)]
            xjobs.reverse()          # pop() from the front via .pop()
            xs_map = {}
            pending = []
            for t in range(T):
                for half in range(2):
                    sg = [gate_pair(t, half, gp) for gp in range(4)]
                    if pending:
                        pending.pop(0)()
                    for _ in range(3):
                        if not xjobs:
                            break
                        tt, hf = xjobs.pop()
                        if tt not in xs_map:
                            xs_map[tt] = load_x(tt)
                        xmix(tt, hf, xs_map[tt])
                    hC = cell(t, half, sg)
                    transposes(half, hC, fill=2)
                    if t < T - 1:
                        pending.append(lambda h=half: hmix(h))
            for job in pending:
                job()

            # ---- final projection ---------------------------------------
            nc.sync.dma_start(ylT.ap()[0], ylt[:, 0, :])
            nc.sync.dma_start(ylT.ap()[1], ylt[:, 1, :])
            ym = tmp.tile([128, 2, ROWS], fp16, tag="ym", name="ym", bufs=1)
            for half in range(2):
                lo, hi, c0 = HALVES[half]
                pm = ps_tile()
                for k in range(NH):
                    for c in range(HCH):
                        nc.tensor.matmul(
                            pm[:, k, c * CROWS:(c + 1) * CROWS],
                            hrow[:, k, c0 + c, :], bd(3),
                            start=True, stop=True)
                nc.vector.tensor_copy(ym[:, :, lo:hi], pm[:, :, :HROWS])
            for ot in range(NH):
                ps = ps_tile()
                ots = tmp.tile([128, ROWS], fp32, tag="ots", name="ots",
                               bufs=2)
                for half in range(2):
                    lo, hi, _ = HALVES[half]
                    dst = ps[:, half, :HROWS]
                    for k in range(NH):
                        nc.tensor.matmul(
                            dst, wfc_s[:, k, ot * 128:(ot + 1) * 128],
                            ym[:, k, lo:hi],
                            start=(k == 0),
                            stop=(k == NH - 1) and not nz["bfc"])
                    if nz["bfc"]:
                        nc.tensor.matmul(
                            dst,
                            bvec_s[:, 3 * G4 + ot * 128:3 * G4 + (ot + 1) * 128],
                            rs(3)[:, lo:hi], start=False, stop=True)
                    nc.scalar.activation(ots[:, lo:hi], dst, AF.Tanh)
                nc.sync.dma_start(outT.ap()[ot], ots[:])

    nc.compile()
    return nc


_CACHE = {}


def kernel(x, G_h1, W_h1, b_h1, G_h2, W_h2, b_h2,
           G_lstm, W_ih, b_ih, W_hh, b_hh, G_fc, W_fc, b_fc):
    from concourse.bass_utils import run_bass_kernel_spmd

    x = np.asarray(x, np.float32)
    gl = _norm_rows(np.asarray(G_lstm, np.float32))
    g1 = _norm_rows(np.asarray(G_h1, np.float32))
    g2 = _norm_rows(np.asarray(G_h2, np.float32))
    gfc = _norm_rows(np.asarray(G_fc, np.float32))

    w_ih = np.asarray(W_ih, np.float32)[:, _GATE_PERM]
    w_hh = np.asarray(W_hh, np.float32)[:, _GATE_PERM]
    bg = (np.asarray(b_ih, np.float32) + np.asarray(b_hh, np.float32))[_GATE_PERM]

    nz = {
        "bg": bool(np.any(bg != 0)),
        "b12": [bool(np.any(np.asarray(b, np.float32) != 0))
                for b in (b_h1, b_h2)],
        "bfc": bool(np.any(np.asarray(b_fc, np.float32) != 0)),
    }
    key = (nz["bg"], tuple(nz["b12"]), nz["bfc"])
    if key not in _CACHE:
        _CACHE[key] = _build(nz)
    nc = _CACHE[key]

    def rs_row(gn):   # [ROWS], natural row order
        return np.tile(np.sum(gn, axis=1), BLOC).astype(np.float32)

    def pad_g4(b):
        v = np.zeros(G4, np.float32)
        b = np.asarray(b, np.float32)
        v[:b.shape[0]] = b
        return v

    bds = np.concatenate(
        [_bd_pad_T(g) for g in (gl, g1, g2, gfc)] + [_bd_pad_T(gl) * S8],
        axis=1).astype(F16)
    bvec = np.concatenate([bg * SGATE, pad_g4(b_h1), pad_g4(b_h2),
                           pad_g4(b_fc)])[None, :].astype(np.float32)
    rsv = np.concatenate([rs_row(g) for g in (gl, g1, g2, gfc)])[None, :]

    wh12 = np.concatenate([np.asarray(W_h1, np.float32),
                           np.asarray(W_h2, np.float32)], axis=1)

    shared = {
        "wih": (w_ih * SGATE).astype(F16),
        "whh8": np.ascontiguousarray(
            (w_hh * S8).reshape(2, 128, G4)).astype(F8)
            .transpose(1, 0, 2).copy(),
        "wh12": wh12.astype(F16),
        "wfc": np.ascontiguousarray(
            np.asarray(W_fc, np.float32).reshape(NH, 128, H)
            .transpose(1, 0, 2)).astype(F16),
        "bds": bds,
        "ident": np.eye(128, dtype=F16),
        "bvec": bvec,
        "rsv": np.ascontiguousarray(rsv.astype(np.float32)),
    }

    in_maps = []
    for core in range(NCORES):
        xs = x[core * BLOC:(core + 1) * BLOC]               # [32, T, N, D]
        xr = xs.transpose(1, 0, 2, 3).reshape(T, ROWS, D)   # natural rows
        xch = np.zeros((T, 128, NCHUNK, D), np.float32)
        xv = xr.reshape(T, NCHUNK, CROWS, D)
        xch[:, :CROWS] = xv.transpose(0, 2, 1, 3)           # [t, slot, c, f]
        m = dict(shared)
        m["xc"] = np.ascontiguousarray(xch.astype(F16))
        in_maps.append(m)

    res = run_bass_kernel_spmd(nc, in_maps, core_ids=list(range(NCORES)))
    kernel.last_results = res
    kernel.last_nc = nc
    kernel.last_in_maps = in_maps

    out = np.empty((B, N, H), np.float32)
    yl = np.empty((B, N, H), np.float32)
    for core in range(NCORES):
        r = res.results[core]
        o = r["outT"].reshape(H, ROWS).T.reshape(BLOC, N, H)
        y = r["ylT"].reshape(H, ROWS).T.reshape(BLOC, N, H)
        out[core * BLOC:(core + 1) * BLOC] = o
        yl[core * BLOC:(core + 1) * BLOC] = y
    return out, yl


# revision 3
# speedup vs baseline: 1.2666x; 1.0238x over previous
"""Trainium2 Bass kernel for nn_Encoder (graph-LSTM encoder over 21 nodes), v2.

Reference model:
  h0 = Gn_h1 @ (x0 W_h1 + b_h1); c0 = Gn_h2 @ (x0 W_h2 + b_h2)
  step t: gates = Gn_lstm @ (x_t W_ih + b_ih + h W_hh + b_hh)
          i,f,g,o = split(gates); c = s(f)*c + s(i)*tanh(g); h = s(o)*tanh(c)
  out = tanh(Gn_fc @ (h_63 W_fc + b_fc)); returns (out, h_63)

Sharding: data-parallel over batch, B=256 -> 32 per core on 8 NeuronCores.

v2 structure (vs v1): rows are split into two independent half-streams
(16 batches each) that software-pipeline the serial per-step chain
(gates -> sigma -> cell -> transpose -> graph-mix -> next gates) across
engines, keeping the PE continuously fed (p-state) and the ACT engine
(the throughput wall: 10 tile-activations/step) near 100% duty.  The
recurrent h-part of the gate matmul runs as a single fp8e4 DoubleRow
matmul (K=256 in one instruction at 0.5 cyc/row); everything else is
fp16.  Gate PSUM tiles hold both feature halves of one gate ([128, 2,
512] = 2 banks) so each sigma/tanh covers 672 free elements in one
instruction.  The graph mix for slot s is emitted one slot later so its
transpose latency hides behind the other half's gate/activation work.
"""

import numpy as np
import ml_dtypes

B, T, N, D, H = 256, 64, 21, 128, 256
NCORES = 8
BLOC = B // NCORES           # 32
ROWS = BLOC * N              # 672
CB = 4                       # batches per chunk
CROWS = CB * N               # 84 slots per chunk
NCHUNK = BLOC // CB          # 8
G4 = 4 * H
NH = H // 128                # 2
HROWS = ROWS // 2            # 336 rows per half
HCH = NCHUNK // 2            # 4 chunks per half
F16 = np.float16
F8 = ml_dtypes.float8_e4m3
LA = 3                       # x prefetch lookahead (steps)
# fp8 range scaling: W_hh and hm are each scaled by 8 (their typical
# magnitudes ~0.05 and ~0.3 would otherwise land in e4m3 subnormals);
# the x-part weights carry the matching x64 so the whole gate PSUM is
# x64, undone exactly by the activation's scale=1/64.
S8 = 8.0
SGATE = S8 * S8

# gate order (i,f,g,o) -> (f,g,i,o): sigma(f) first unblocks q = f*c
# early, and the late consumers (p needs i, h needs o) come last, which
# shortens the recurrent dependency chain through the ACT stream.
_GATE_PERM = np.concatenate([
    np.arange(H, 2 * H),                # f
    np.arange(2 * H, 3 * H),            # g
    np.arange(0, H),                    # i
    np.arange(3 * H, 4 * H)])           # o
# per-gate activation: g=tanh, rest sigmoid
_GATE_FUNCS = ("sigmoid", "tanh", "sigmoid", "sigmoid")


def _norm_rows(g):
    return g / np.clip(np.sum(np.abs(g), axis=1, keepdims=True), 1e-12, None)


def _bd_pad_T(gn):
    """[128, CROWS]: bd[j_slot, i_slot] = G[i_node, j_node] per batch block."""
    bd = np.zeros((128, CROWS), np.float32)
    for bb in range(CB):
        s = bb * N
        bd[s:s + N, s:s + N] = gn.T
    return bd


def _build(nz):
    import concourse.bass as bass
    import concourse.bacc as bacc
    import concourse.mybir as mybir
    import concourse.tile as tile

    fp32 = mybir.dt.float32
    fp16 = mybir.dt.float16
    fp8 = mybir.dt.float8e4
    AF = mybir.ActivationFunctionType
    DR = mybir.MatmulPerfMode.DoubleRowSwInterleave
    ALU = mybir.AluOpType

    nc = bacc.Bacc("TRN2", target_bir_lowering=False, debug=False,
                   num_devices=NCORES)

    xc = nc.dram_tensor("xc", [T, 128, NCHUNK, D], fp16, kind="ExternalInput")
    wih = nc.dram_tensor("wih", [D, G4], fp16, kind="ExternalInput")
    # W_hh packed for DoubleRowSwInterleave: per gate tile, k-tile pairs
    # interleaved per column with columns reversed (see bass_interp).
    whh8 = nc.dram_tensor("whh8", [128, 8, 256], fp8, kind="ExternalInput")
    wh12 = nc.dram_tensor("wh12", [D, 2 * H], fp16, kind="ExternalInput")
    wfc = nc.dram_tensor("wfc", [128, 2, H], fp16, kind="ExternalInput")
    bds = nc.dram_tensor("bds", [128, 5 * CROWS], fp16, kind="ExternalInput")
    ident = nc.dram_tensor("ident", [128, 128], fp16, kind="ExternalInput")
    bvec = nc.dram_tensor("bvec", [1, 4 * G4], fp32, kind="ExternalInput")
    rsv = nc.dram_tensor("rsv", [1, 4 * ROWS], fp32, kind="ExternalInput")

    outT = nc.dram_tensor("outT", [NH, 128, ROWS], fp32, kind="ExternalOutput")
    ylT = nc.dram_tensor("ylT", [NH, 128, ROWS], fp32, kind="ExternalOutput")

    HALVES = ((0, HROWS, 0), (HROWS, ROWS, HCH))   # (lo, hi, chunk0)

    with tile.TileContext(nc) as tc:
        with (
            tc.tile_pool(name="const", bufs=1) as const,
            tc.tile_pool(name="state", bufs=1) as state,
            tc.tile_pool(name="xstage", bufs=8) as xstage,
            tc.tile_pool(name="acts", bufs=12) as acts,
            tc.tile_pool(name="tmp", bufs=8) as tmp,
            tc.tile_pool(name="psum", bufs=3, space="PSUM") as psum,
        ):
            def load_const(dram, shape, dt):
                t = const.tile(shape, dt, tag=dram.name, name=dram.name + "_s")
                nc.sync.dma_start(t[:], dram.ap())
                return t

            wih_s = load_const(wih, [D, G4], fp16)
            whh_s = load_const(whh8, [128, 8, 256], fp8)
            wh12_s = load_const(wh12, [D, 2 * H], fp16)
            wfc_s = load_const(wfc, [128, 2, H], fp16)
            bds_s = load_const(bds, [128, 5 * CROWS], fp16)
            ident_s = load_const(ident, [128, 128], fp16)
            any_bias = nz["bg"] or any(nz["b12"]) or nz["bfc"]
            if any_bias:
                bvec_s = load_const(bvec, [1, 4 * G4], fp32)
                rsv_s = load_const(rsv, [1, 4 * ROWS], fp32)

            def bd(i):
                return bds_s[:, i * CROWS:(i + 1) * CROWS]

            def rs(i):
                return rsv_s[:, i * ROWS:(i + 1) * ROWS]

            xmT = state.tile([128, T, ROWS], fp16, tag="xmT", name="xmT")
            hmT = state.tile([128, 2, ROWS], fp8, tag="hmT", name="hmT")
            cT = state.tile([128, 2, ROWS], fp16, tag="cT", name="cT")
            # transposed h: [slot, fh, chunk, feat]; pad slots 84:128 are
            # zeroed once and never written (the PE transposes + copies only
            # touch 0:84), so the graph-mix contraction over the full 128
            # partitions stays clean.
            hrow = state.tile([128, 2, NCHUNK, 128], fp16, tag="hrow",
                              name="hrow")
            ylt = state.tile([128, 2, ROWS], fp32, tag="ylt", name="ylt")

            nc.vector.memset(hrow[:], 0.0)

            def ps_tile():
                return psum.tile([128, 2, 512], fp32, tag="ps", name="ps")

            def pst_tile():
                return psum.tile([128, 2, HCH, 128], fp16, tag="pst",
                                 name="pst", bufs=2)

            def cvw(ap_2d, nch=HCH):
                """[128, nch*84] compact -> [128, c, s] view."""
                return ap_2d.rearrange("p (c s) -> p c s", s=CROWS)

            # ---- building blocks ----------------------------------------
            def mix_to(ps_half, src_chunks, bd_ap, c0, nch=HCH):
                """mix chunks c0..c0+nch of src into ps_half[:, c*84:...]."""
                for c in range(nch):
                    nc.tensor.matmul(
                        ps_half[:, c * CROWS:(c + 1) * CROWS],
                        src_chunks[:, c0 + c, :], bd_ap, start=True, stop=True)

            def load_x(t):
                xs = xstage.tile([128, NCHUNK, D], fp16, tag="xs", name="xs")
                nc.sync.dma_start(xs[:], xc.ap()[t])
                return xs

            def xmix(t, half, xs):
                lo, hi, c0 = HALVES[half]
                ps = ps_tile()
                mix_to(ps[:, 0, :], xs[:], bd(0), c0)
                nc.vector.tensor_copy(xmT[:, t, lo:hi], ps[:, 0, :HROWS])

            def gate_pair(t, half, gp):
                """one gate-pair matmul group + paired activation."""
                lo, hi, _ = HALVES[half]
                ps = ps_tile()
                for j in range(NH):
                    gt = 2 * gp + j
                    gsl = slice(gt * 128, (gt + 1) * 128)
                    nc.tensor.matmul(ps[:, j, :HROWS], wih_s[:, gsl],
                                     xmT[:, t, lo:hi],
                                     start=True, stop=False)
                    nc.tensor.matmul(ps[:, j, :HROWS],
                                     whh_s[:, gt, :], hmT[:, :, lo:hi],
                                     perf_mode=DR,
                                     start=False, stop=not nz["bg"])
                    if nz["bg"]:
                        nc.tensor.matmul(ps[:, j, :HROWS], bvec_s[:, gsl],
                                         rs(0)[:, lo:hi],
                                         start=False, stop=True)
                o = acts.tile([128, 2, HROWS], fp16, tag="sg", name="sg")
                nc.scalar.activation(
                    o[:], ps[:, :, :HROWS],
                    AF.Tanh if _GATE_FUNCS[gp] == "tanh" else AF.Sigmoid,
                    scale=1.0 / SGATE)
                return o

            def cell(t, half, sg):
                """c' = s(f)c + s(i)tanh(g); h = s(o)tanh(c')."""
                lo, hi, c0 = HALVES[half]
                sf, tg, si, so = sg
                with tc.high_priority():
                    q = tmp.tile([128, 2, HROWS], fp16, tag="q", name="q",
                                 bufs=3)
                    nc.gpsimd.tensor_mul(q[:], sf[:], cT[:, :, lo:hi])
                    p = tmp.tile([128, 2, HROWS], fp16, tag="p", name="p",
                                 bufs=3)
                    nc.vector.tensor_mul(p[:], si[:], tg[:])
                    nc.vector.tensor_add(cT[:, :, lo:hi], q[:], p[:])
                    tc_ = tmp.tile([128, 2, HROWS], fp16, tag="tc", name="tc",
                                   bufs=3)
                    nc.scalar.activation(tc_[:], cT[:, :, lo:hi], AF.Tanh)
                    hC = tmp.tile([128, 2, HROWS], fp16, tag="hC", name="hC",
                                  bufs=3)
                    nc.vector.scalar_tensor_tensor(
                        hC[:], so[:], 1.0, tc_[:], ALU.mult, ALU.mult)
                if t == T - 1:
                    for k in range(NH):
                        nc.vector.tensor_mul(ylt[:, k, lo:hi],
                                             so[:, k, :], tc_[:, k, :])
                return hC

            def transposes(half, hC, fill=0):
                """PE-transpose h chunks into PSUM, then one copy to hrow."""
                _, _, c0 = HALVES[half]
                with tc.high_priority():
                    pst = pst_tile()
                    for k in range(NH):
                        for c in range(HCH):
                            nc.tensor.transpose(
                                pst[:CROWS, k, c, :],
                                hC[:, k, c * CROWS:(c + 1) * CROWS],
                                ident_s[:])
                    nc.vector.tensor_copy(hrow[:CROWS, :, c0:c0 + HCH, :],
                                          pst[:CROWS, :, :, :])
                for _ in range(fill):
                    pf = pst_tile()
                    nc.tensor.transpose(pf[:CROWS, 0, 0, :],
                                        hC[:, 0, 0:CROWS], ident_s[:])

            def hmix(half):
                """graph-mix h (slot-major in hrow) -> hmT feature-major."""
                lo, hi, c0 = HALVES[half]
                with tc.high_priority():
                    ps = ps_tile()
                    for k in range(NH):
                        for c in range(HCH):
                            nc.tensor.matmul(
                                ps[:, k, c * CROWS:(c + 1) * CROWS],
                                hrow[:, k, c0 + c, :], bd(4),
                                start=True, stop=True)
                        nc.vector.tensor_copy(hmT[:, k, lo:hi],
                                              ps[:, k, :HROWS])

            # ---- init: h0 / c0 ------------------------------------------
            xs0 = load_x(0)
            for k12 in range(2):
                pm = ps_tile()
                for half in range(2):
                    mix_to(pm[:, half, :], xs0[:], bd(1 + k12), HALVES[half][2])
                xm12 = tmp.tile([128, ROWS], fp16, tag="xm12", name="xm12",
                                bufs=1)
                for half in range(2):
                    lo, hi, _ = HALVES[half]
                    nc.vector.tensor_copy(xm12[:, lo:hi], pm[:, half, :HROWS])
                for half in range(2):
                    lo, hi, c0 = HALVES[half]
                    po = ps_tile()
                    for j in range(NH):
                        wsl = wh12_s[:, k12 * H + j * 128:
                                     k12 * H + (j + 1) * 128]
                        nc.tensor.matmul(po[:, j, :HROWS], wsl, xm12[:, lo:hi],
                                         start=True, stop=not nz["b12"][k12])
                        if nz["b12"][k12]:
                            nc.tensor.matmul(
                                po[:, j, :HROWS],
                                bvec_s[:, (1 + k12) * G4 + j * 128:
                                       (1 + k12) * G4 + (j + 1) * 128],
                                rs(1 + k12)[:, lo:hi], start=False, stop=True)
                    if k12 == 0:
                        hC0 = tmp.tile([128, 2, HROWS], fp16, tag="hC",
                                       name="hC0", bufs=3)
                        nc.vector.tensor_copy(hC0[:], po[:, :, :HROWS])
                        transposes(half, hC0)
                    else:
                        nc.vector.tensor_copy(cT[:, :, lo:hi],
                                              po[:, :, :HROWS])

            for half in range(2):
                hmix(half)
            for tt in range(min(LA, T)):
                xs = xs0 if tt == 0 else load_x(tt)
                for half in range(2):
                    xmix(tt, half, xs)

            # ---- recurrent loop: half-slot software pipeline -------------
            # PE stream per slot: 4 gate pairs, the PREVIOUS slot's
            # graph-mix (whose transposed h landed mid-previous-slot), up
            # to 3 run-ahead x-mix jobs (real work that doubles as p-state
            # filler), this slot's h PE-transposes, then dummy filler
            # transposes.  The mix result is consumed by the same half's
            # gates one full slot later.
            xjobs = [(tt, hf) for tt in range(LA, T) for hf in range(2)]
            xjobs.reverse()          # pop() from the front via .pop()
            xs_map = {}
            pending = []
            for t in range(T):
                for half in range(2):
                    sg = [gate_pair(t, half, gp) for gp in range(4)]
                    if pending:
                        pending.pop(0)()
                    for _ in range(3):
                        if not xjobs:
                            break
                        tt, hf = xjobs.pop()
                        if tt not in xs_map:
                            xs_map[tt] = load_x(tt)
                        xmix(tt, hf, xs_map[tt])
                    hC = cell(t, half, sg)
                    transposes(half, hC, fill=2)
                    if t < T - 1:
                        pending.append(lambda h=half: hmix(h))
            for job in pending:
                job()

            # ---- final projection ---------------------------------------
            nc.sync.dma_start(ylT.ap()[0], ylt[:, 0, :])
            nc.sync.dma_start(ylT.ap()[1], ylt[:, 1, :])
            ym = tmp.tile([128, 2, ROWS], fp16, tag="ym", name="ym", bufs=1)
            for half in range(2):
                lo, hi, c0 = HALVES[half]
                pm = ps_tile()
                for k in range(NH):
                    for c in range(HCH):
                        nc.tensor.matmul(
                            pm[:, k, c * CROWS:(c + 1) * CROWS],
                            hrow[:, k, c0 + c, :], bd(3),
                            start=True, stop=True)
                nc.vector.tensor_copy(ym[:, :, lo:hi], pm[:, :, :HROWS])
            for ot in range(NH):
                ps = ps_tile()
                ots = tmp.tile([128, ROWS], fp32, tag="ots", name="ots",
                               bufs=2)
                for half in range(2):
                    lo, hi, _ = HALVES[half]
                    dst = ps[:, half, :HROWS]
                    for k in range(NH):
                        nc.tensor.matmul(
                            dst, wfc_s[:, k, ot * 128:(ot + 1) * 128],
                            ym[:, k, lo:hi],
                            start=(k == 0),
                            stop=(k == NH - 1) and not nz["bfc"])
                    if nz["bfc"]:
                        nc.tensor.matmul(
                            dst,
                            bvec_s[:, 3 * G4 + ot * 128:3 * G4 + (ot + 1) * 128],
                            rs(3)[:, lo:hi], start=False, stop=True)
                    nc.scalar.activation(ots[:, lo:hi], dst, AF.Tanh)
                nc.sync.dma_start(outT.ap()[ot], ots[:])

    nc.compile()
    return nc


_CACHE = {}


def kernel(x, G_h1, W_h1, b_h1, G_h2, W_h2, b_h2,
           G_lstm, W_ih, b_ih, W_hh, b_hh, G_fc, W_fc, b_fc):
    from concourse.bass_utils import run_bass_kernel_spmd

    x = np.asarray(x, np.float32)
    gl = _norm_rows(np.asarray(G_lstm, np.float32))
    g1 = _norm_rows(np.asarray(G_h1, np.float32))
    g2 = _norm_rows(np.asarray(G_h2, np.float32))
    gfc = _norm_rows(np.asarray(G_fc, np.float32))

    w_ih = np.asarray(W_ih, np.float32)[:, _GATE_PERM]
    w_hh = np.asarray(W_hh, np.float32)[:, _GATE_PERM]
    bg = (np.asarray(b_ih, np.float32) + np.asarray(b_hh, np.float32))[_GATE_PERM]

    nz = {
        "bg": bool(np.any(bg != 0)),
        "b12": [bool(np.any(np.asarray(b, np.float32) != 0))
                for b in (b_h1, b_h2)],
        "bfc": bool(np.any(np.asarray(b_fc, np.float32) != 0)),
    }
    key = (nz["bg"], tuple(nz["b12"]), nz["bfc"])
    if key not in _CACHE:
        _CACHE[key] = _build(nz)
    nc = _CACHE[key]

    def rs_row(gn):   # [ROWS], natural row order
        return np.tile(np.sum(gn, axis=1), BLOC).astype(np.float32)

    def pad_g4(b):
        v = np.zeros(G4, np.float32)
        b = np.asarray(b, np.float32)
        v[:b.shape[0]] = b
        return v

    bds = np.concatenate(
        [_bd_pad_T(g) for g in (gl, g1, g2, gfc)] + [_bd_pad_T(gl) * S8],
        axis=1).astype(F16)
    bvec = np.concatenate([bg * SGATE, pad_g4(b_h1), pad_g4(b_h2),
                           pad_g4(b_fc)])[None, :].astype(np.float32)
    rsv = np.concatenate([rs_row(g) for g in (gl, g1, g2, gfc)])[None, :]

    wh12 = np.concatenate([np.asarray(W_h1, np.float32),
                           np.asarray(W_h2, np.float32)], axis=1)

    # DoubleRowSwInterleave weight packing: per gate tile gt the 256-col
    # block holds stored[:, 2j+i] = W_ktile_i[:, 127-j] (interp semantics:
    # deinterleave then reverse columns restores the logical weights).
    w8 = (w_hh * S8).astype(F8).astype(np.float32)      # quantize first
    w8 = w8.reshape(2, 128, 8, 128)[:, :, :, ::-1]      # [i, p, gt, jrev]
    whh_swi = np.ascontiguousarray(
        w8.transpose(1, 2, 3, 0)).astype(F8)            # [p, gt, j, i]

    shared = {
        "wih": (w_ih * SGATE).astype(F16),
        "whh8": whh_swi.reshape(128, 8, 256),
        "wh12": wh12.astype(F16),
        "wfc": np.ascontiguousarray(
            np.asarray(W_fc, np.float32).reshape(NH, 128, H)
            .transpose(1, 0, 2)).astype(F16),
        "bds": bds,
        "ident": np.eye(128, dtype=F16),
        "bvec": bvec,
        "rsv": np.ascontiguousarray(rsv.astype(np.float32)),
    }

    in_maps = []
    for core in range(NCORES):
        xs = x[core * BLOC:(core + 1) * BLOC]               # [32, T, N, D]
        xr = xs.transpose(1, 0, 2, 3).reshape(T, ROWS, D)   # natural rows
        xch = np.zeros((T, 128, NCHUNK, D), np.float32)
        xv = xr.reshape(T, NCHUNK, CROWS, D)
        xch[:, :CROWS] = xv.transpose(0, 2, 1, 3)           # [t, slot, c, f]
        m = dict(shared)
        m["xc"] = np.ascontiguousarray(xch.astype(F16))
        in_maps.append(m)

    res = run_bass_kernel_spmd(nc, in_maps, core_ids=list(range(NCORES)))
    kernel.last_results = res
    kernel.last_nc = nc
    kernel.last_in_maps = in_maps

    out = np.empty((B, N, H), np.float32)
    yl = np.empty((B, N, H), np.float32)
    for core in range(NCORES):
        r = res.results[core]
        o = r["outT"].reshape(H, ROWS).T.reshape(BLOC, N, H)
        y = r["ylT"].reshape(H, ROWS).T.reshape(BLOC, N, H)
        out[core * BLOC:(core + 1) * BLOC] = o
        yl[core * BLOC:(core + 1) * BLOC] = y
    return out, yl


# revision 4
# speedup vs baseline: 1.2677x; 1.0009x over previous
"""Trainium2 Bass kernel for nn_Encoder (graph-LSTM encoder over 21 nodes), v2.

Reference model:
  h0 = Gn_h1 @ (x0 W_h1 + b_h1); c0 = Gn_h2 @ (x0 W_h2 + b_h2)
  step t: gates = Gn_lstm @ (x_t W_ih + b_ih + h W_hh + b_hh)
          i,f,g,o = split(gates); c = s(f)*c + s(i)*tanh(g); h = s(o)*tanh(c)
  out = tanh(Gn_fc @ (h_63 W_fc + b_fc)); returns (out, h_63)

Sharding: data-parallel over batch, B=256 -> 32 per core on 8 NeuronCores.

v2 structure (vs v1): rows are split into two independent half-streams
(16 batches each) that software-pipeline the serial per-step chain
(gates -> sigma -> cell -> transpose -> graph-mix -> next gates) across
engines, keeping the PE continuously fed (p-state) and the ACT engine
(the throughput wall: 10 tile-activations/step) near 100% duty.  The
recurrent h-part of the gate matmul runs as a single fp8e4 DoubleRow
matmul (K=256 in one instruction at 0.5 cyc/row); everything else is
fp16.  Gate PSUM tiles hold both feature halves of one gate ([128, 2,
512] = 2 banks) so each sigma/tanh covers 672 free elements in one
instruction.  The graph mix for slot s is emitted one slot later so its
transpose latency hides behind the other half's gate/activation work.
"""

import numpy as np
import ml_dtypes

B, T, N, D, H = 256, 64, 21, 128, 256
NCORES = 8
BLOC = B // NCORES           # 32
ROWS = BLOC * N              # 672
CB = 4                       # batches per chunk
CROWS = CB * N               # 84 slots per chunk
NCHUNK = BLOC // CB          # 8
G4 = 4 * H
NH = H // 128                # 2
HROWS = ROWS // 2            # 336 rows per half
HCH = NCHUNK // 2            # 4 chunks per half
F16 = np.float16
F8 = ml_dtypes.float8_e4m3
LA = 3                       # x prefetch lookahead (steps)
# fp8 range scaling: W_hh and hm are each scaled by 8 (their typical
# magnitudes ~0.05 and ~0.3 would otherwise land in e4m3 subnormals);
# the x-part weights carry the matching x64 so the whole gate PSUM is
# x64, undone exactly by the activation's scale=1/64.
S8 = 8.0
SGATE = S8 * S8

# gate order (i,f,g,o) -> (f,g,i,o): sigma(f) first unblocks q = f*c
# early, and the late consumers (p needs i, h needs o) come last, which
# shortens the recurrent dependency chain through the ACT stream.
_GATE_PERM = np.concatenate([
    np.arange(H, 2 * H),                # f
    np.arange(2 * H, 3 * H),            # g
    np.arange(0, H),                    # i
    np.arange(3 * H, 4 * H)])           # o
# per-gate activation: g=tanh, rest sigmoid
_GATE_FUNCS = ("sigmoid", "tanh", "sigmoid", "sigmoid")


def _norm_rows(g):
    return g / np.clip(np.sum(np.abs(g), axis=1, keepdims=True), 1e-12, None)


def _bd_pad_T(gn):
    """[128, CROWS]: bd[j_slot, i_slot] = G[i_node, j_node] per batch block."""
    bd = np.zeros((128, CROWS), np.float32)
    for bb in range(CB):
        s = bb * N
        bd[s:s + N, s:s + N] = gn.T
    return bd


def _build(nz):
    import concourse.bass as bass
    import concourse.bacc as bacc
    import concourse.mybir as mybir
    import concourse.tile as tile

    fp32 = mybir.dt.float32
    fp16 = mybir.dt.float16
    fp8 = mybir.dt.float8e4
    AF = mybir.ActivationFunctionType
    DR = mybir.MatmulPerfMode.DoubleRowSwInterleave
    ALU = mybir.AluOpType

    nc = bacc.Bacc("TRN2", target_bir_lowering=False, debug=False,
                   num_devices=NCORES)

    xc = nc.dram_tensor("xc", [T, 128, NCHUNK, D], fp16, kind="ExternalInput")
    wih = nc.dram_tensor("wih", [D, G4], fp16, kind="ExternalInput")
    # W_hh packed for DoubleRowSwInterleave: per gate tile, k-tile pairs
    # interleaved per column with columns reversed (see bass_interp).
    whh8 = nc.dram_tensor("whh8", [128, 8, 256], fp8, kind="ExternalInput")
    wh12 = nc.dram_tensor("wh12", [D, 2 * H], fp16, kind="ExternalInput")
    wfc = nc.dram_tensor("wfc", [128, 2, H], fp16, kind="ExternalInput")
    bds = nc.dram_tensor("bds", [128, 5 * CROWS], fp16, kind="ExternalInput")
    ident = nc.dram_tensor("ident", [128, 128], fp16, kind="ExternalInput")
    bvec = nc.dram_tensor("bvec", [1, 4 * G4], fp32, kind="ExternalInput")
    rsv = nc.dram_tensor("rsv", [1, 4 * ROWS], fp32, kind="ExternalInput")

    outT = nc.dram_tensor("outT", [NH, 128, ROWS], fp32, kind="ExternalOutput")
    ylT = nc.dram_tensor("ylT", [NH, 128, ROWS], fp32, kind="ExternalOutput")

    HALVES = ((0, HROWS, 0), (HROWS, ROWS, HCH))   # (lo, hi, chunk0)

    with tile.TileContext(nc) as tc:
        with (
            tc.tile_pool(name="const", bufs=1) as const,
            tc.tile_pool(name="state", bufs=1) as state,
            tc.tile_pool(name="xstage", bufs=8) as xstage,
            tc.tile_pool(name="acts", bufs=12) as acts,
            tc.tile_pool(name="tmp", bufs=8) as tmp,
            tc.tile_pool(name="psum", bufs=3, space="PSUM") as psum,
        ):
            def load_const(dram, shape, dt):
                t = const.tile(shape, dt, tag=dram.name, name=dram.name + "_s")
                nc.sync.dma_start(t[:], dram.ap())
                return t

            wih_s = load_const(wih, [D, G4], fp16)
            whh_s = load_const(whh8, [128, 8, 256], fp8)
            wh12_s = load_const(wh12, [D, 2 * H], fp16)
            wfc_s = load_const(wfc, [128, 2, H], fp16)
            bds_s = load_const(bds, [128, 5 * CROWS], fp16)
            ident_s = load_const(ident, [128, 128], fp16)
            any_bias = nz["bg"] or any(nz["b12"]) or nz["bfc"]
            if any_bias:
                bvec_s = load_const(bvec, [1, 4 * G4], fp32)
                rsv_s = load_const(rsv, [1, 4 * ROWS], fp32)

            def bd(i):
                return bds_s[:, i * CROWS:(i + 1) * CROWS]

            def rs(i):
                return rsv_s[:, i * ROWS:(i + 1) * ROWS]

            xmT = state.tile([128, T, ROWS], fp16, tag="xmT", name="xmT")
            hmT = state.tile([128, 2, ROWS], fp8, tag="hmT", name="hmT")
            cT = state.tile([128, 2, ROWS], fp16, tag="cT", name="cT")
            # transposed h: [slot, fh, chunk, feat]; pad slots 84:128 are
            # zeroed once and never written (the PE transposes + copies only
            # touch 0:84), so the graph-mix contraction over the full 128
            # partitions stays clean.
            hrow = state.tile([128, 2, NCHUNK, 128], fp16, tag="hrow",
                              name="hrow")
            ylt = state.tile([128, 2, ROWS], fp32, tag="ylt", name="ylt")

            nc.vector.memset(hrow[:], 0.0)

            def ps_tile():
                return psum.tile([128, 2, 512], fp32, tag="ps", name="ps")

            def pst_tile():
                return psum.tile([128, 2, HCH, 128], fp16, tag="pst",
                                 name="pst", bufs=2)

            def cvw(ap_2d, nch=HCH):
                """[128, nch*84] compact -> [128, c, s] view."""
                return ap_2d.rearrange("p (c s) -> p c s", s=CROWS)

            # ---- building blocks ----------------------------------------
            def mix_to(ps_half, src_chunks, bd_ap, c0, nch=HCH):
                """mix chunks c0..c0+nch of src into ps_half[:, c*84:...]."""
                for c in range(nch):
                    nc.tensor.matmul(
                        ps_half[:, c * CROWS:(c + 1) * CROWS],
                        src_chunks[:, c0 + c, :], bd_ap, start=True, stop=True)

            def load_x(t):
                xs = xstage.tile([128, NCHUNK, D], fp16, tag="xs", name="xs")
                nc.sync.dma_start(xs[:], xc.ap()[t])
                return xs

            def xmix(t, xs):
                """x graph-mix for one timestep, both halves fused: 8 mix
                matmuls into one PSUM tile, one 672-wide cast out."""
                ps = ps_tile()
                for half in range(2):
                    mix_to(ps[:, half, :], xs[:], bd(0), HALVES[half][2])
                nc.vector.tensor_copy(
                    xmT[:, t, :].rearrange("p (h w) -> p h w", w=HROWS),
                    ps[:, :, :HROWS])

            def gate_pair(t, half, gp):
                """one gate-pair matmul group + paired activation."""
                lo, hi, _ = HALVES[half]
                ps = ps_tile()
                for j in range(NH):
                    gt = 2 * gp + j
                    gsl = slice(gt * 128, (gt + 1) * 128)
                    nc.tensor.matmul(ps[:, j, :HROWS], wih_s[:, gsl],
                                     xmT[:, t, lo:hi],
                                     start=True, stop=False)
                    nc.tensor.matmul(ps[:, j, :HROWS],
                                     whh_s[:, gt, :], hmT[:, :, lo:hi],
                                     perf_mode=DR,
                                     start=False, stop=not nz["bg"])
                    if nz["bg"]:
                        nc.tensor.matmul(ps[:, j, :HROWS], bvec_s[:, gsl],
                                         rs(0)[:, lo:hi],
                                         start=False, stop=True)
                o = acts.tile([128, 2, HROWS], fp16, tag="sg", name="sg")
                nc.scalar.activation(
                    o[:], ps[:, :, :HROWS],
                    AF.Tanh if _GATE_FUNCS[gp] == "tanh" else AF.Sigmoid,
                    scale=1.0 / SGATE)
                return o

            def cell(t, half, sg):
                """c' = s(f)c + s(i)tanh(g); h = s(o)tanh(c')."""
                lo, hi, c0 = HALVES[half]
                sf, tg, si, so = sg
                with tc.high_priority():
                    q = tmp.tile([128, 2, HROWS], fp16, tag="q", name="q",
                                 bufs=3)
                    nc.gpsimd.tensor_mul(q[:], sf[:], cT[:, :, lo:hi])
                    p = tmp.tile([128, 2, HROWS], fp16, tag="p", name="p",
                                 bufs=3)
                    nc.vector.tensor_mul(p[:], si[:], tg[:])
                    nc.vector.tensor_add(cT[:, :, lo:hi], q[:], p[:])
                    tc_ = tmp.tile([128, 2, HROWS], fp16, tag="tc", name="tc",
                                   bufs=3)
                    nc.scalar.activation(tc_[:], cT[:, :, lo:hi], AF.Tanh)
                    hC = tmp.tile([128, 2, HROWS], fp16, tag="hC", name="hC",
                                  bufs=3)
                    nc.vector.scalar_tensor_tensor(
                        hC[:], so[:], 1.0, tc_[:], ALU.mult, ALU.mult)
                if t == T - 1:
                    for k in range(NH):
                        nc.vector.tensor_mul(ylt[:, k, lo:hi],
                                             so[:, k, :], tc_[:, k, :])
                return hC

            def transposes(half, hC, fill=0):
                """PE-transpose h chunks into PSUM, then one copy to hrow."""
                _, _, c0 = HALVES[half]
                with tc.high_priority():
                    pst = pst_tile()
                    for k in range(NH):
                        for c in range(HCH):
                            nc.tensor.transpose(
                                pst[:CROWS, k, c, :],
                                hC[:, k, c * CROWS:(c + 1) * CROWS],
                                ident_s[:])
                    nc.vector.tensor_copy(hrow[:CROWS, :, c0:c0 + HCH, :],
                                          pst[:CROWS, :, :, :])
                for _ in range(fill):
                    pf = pst_tile()
                    nc.tensor.transpose(pf[:CROWS, 0, 0, :],
                                        hC[:, 0, 0:CROWS], ident_s[:])

            def hmix(half):
                """graph-mix h (slot-major in hrow) -> hmT feature-major."""
                lo, hi, c0 = HALVES[half]
                with tc.high_priority():
                    ps = ps_tile()
                    for k in range(NH):
                        for c in range(HCH):
                            nc.tensor.matmul(
                                ps[:, k, c * CROWS:(c + 1) * CROWS],
                                hrow[:, k, c0 + c, :], bd(4),
                                start=True, stop=True)
                        nc.vector.tensor_copy(hmT[:, k, lo:hi],
                                              ps[:, k, :HROWS])

            # ---- init: h0 / c0 ------------------------------------------
            xs0 = load_x(0)
            for k12 in range(2):
                pm = ps_tile()
                for half in range(2):
                    mix_to(pm[:, half, :], xs0[:], bd(1 + k12), HALVES[half][2])
                xm12 = tmp.tile([128, ROWS], fp16, tag="xm12", name="xm12",
                                bufs=1)
                for half in range(2):
                    lo, hi, _ = HALVES[half]
                    nc.vector.tensor_copy(xm12[:, lo:hi], pm[:, half, :HROWS])
                for half in range(2):
                    lo, hi, c0 = HALVES[half]
                    po = ps_tile()
                    for j in range(NH):
                        wsl = wh12_s[:, k12 * H + j * 128:
                                     k12 * H + (j + 1) * 128]
                        nc.tensor.matmul(po[:, j, :HROWS], wsl, xm12[:, lo:hi],
                                         start=True, stop=not nz["b12"][k12])
                        if nz["b12"][k12]:
                            nc.tensor.matmul(
                                po[:, j, :HROWS],
                                bvec_s[:, (1 + k12) * G4 + j * 128:
                                       (1 + k12) * G4 + (j + 1) * 128],
                                rs(1 + k12)[:, lo:hi], start=False, stop=True)
                    if k12 == 0:
                        hC0 = tmp.tile([128, 2, HROWS], fp16, tag="hC",
                                       name="hC0", bufs=3)
                        nc.vector.tensor_copy(hC0[:], po[:, :, :HROWS])
                        transposes(half, hC0)
                    else:
                        nc.vector.tensor_copy(cT[:, :, lo:hi],
                                              po[:, :, :HROWS])

            for half in range(2):
                hmix(half)
            for tt in range(min(LA, T)):
                xs = xs0 if tt == 0 else load_x(tt)
                xmix(tt, xs)

            # ---- recurrent loop: half-slot software pipeline -------------
            # PE stream per slot: 4 gate pairs, the PREVIOUS slot's
            # graph-mix (whose transposed h landed mid-previous-slot), up
            # to 3 run-ahead x-mix jobs (real work that doubles as p-state
            # filler), this slot's h PE-transposes, then dummy filler
            # transposes.  The mix result is consumed by the same half's
            # gates one full slot later.
            pending = []
            for t in range(T):
                for half in range(2):
                    sg = [gate_pair(t, half, gp) for gp in range(4)]
                    if pending:
                        pending.pop(0)()
                    if half == 0 and t + LA < T:
                        xs_new = load_x(t + LA)
                        xmix(t + LA, xs_new)
                    hC = cell(t, half, sg)
                    transposes(half, hC, fill=2)
                    if t < T - 1:
                        pending.append(lambda h=half: hmix(h))
            for job in pending:
                job()

            # ---- final projection ---------------------------------------
            nc.sync.dma_start(ylT.ap()[0], ylt[:, 0, :])
            nc.sync.dma_start(ylT.ap()[1], ylt[:, 1, :])
            ym = tmp.tile([128, 2, ROWS], fp16, tag="ym", name="ym", bufs=1)
            for half in range(2):
                lo, hi, c0 = HALVES[half]
                pm = ps_tile()
                for k in range(NH):
                    for c in range(HCH):
                        nc.tensor.matmul(
                            pm[:, k, c * CROWS:(c + 1) * CROWS],
                            hrow[:, k, c0 + c, :], bd(3),
                            start=True, stop=True)
                nc.vector.tensor_copy(ym[:, :, lo:hi], pm[:, :, :HROWS])
            for ot in range(NH):
                ps = ps_tile()
                ots = tmp.tile([128, ROWS], fp32, tag="ots", name="ots",
                               bufs=2)
                for half in range(2):
                    lo, hi, _ = HALVES[half]
                    dst = ps[:, half, :HROWS]
                    for k in range(NH):
                        nc.tensor.matmul(
                            dst, wfc_s[:, k, ot * 128:(ot + 1) * 128],
                            ym[:, k, lo:hi],
                            start=(k == 0),
                            stop=(k == NH - 1) and not nz["bfc"])
                    if nz["bfc"]:
                        nc.tensor.matmul(
                            dst,
                            bvec_s[:, 3 * G4 + ot * 128:3 * G4 + (ot + 1) * 128],
                            rs(3)[:, lo:hi], start=False, stop=True)
                    nc.scalar.activation(ots[:, lo:hi], dst, AF.Tanh)
                nc.sync.dma_start(outT.ap()[ot], ots[:])

    nc.compile()
    return nc


_CACHE = {}


def kernel(x, G_h1, W_h1, b_h1, G_h2, W_h2, b_h2,
           G_lstm, W_ih, b_ih, W_hh, b_hh, G_fc, W_fc, b_fc):
    from concourse.bass_utils import run_bass_kernel_spmd

    x = np.asarray(x, np.float32)
    gl = _norm_rows(np.asarray(G_lstm, np.float32))
    g1 = _norm_rows(np.asarray(G_h1, np.float32))
    g2 = _norm_rows(np.asarray(G_h2, np.float32))
    gfc = _norm_rows(np.asarray(G_fc, np.float32))

    w_ih = np.asarray(W_ih, np.float32)[:, _GATE_PERM]
    w_hh = np.asarray(W_hh, np.float32)[:, _GATE_PERM]
    bg = (np.asarray(b_ih, np.float32) + np.asarray(b_hh, np.float32))[_GATE_PERM]

    nz = {
        "bg": bool(np.any(bg != 0)),
        "b12": [bool(np.any(np.asarray(b, np.float32) != 0))
                for b in (b_h1, b_h2)],
        "bfc": bool(np.any(np.asarray(b_fc, np.float32) != 0)),
    }
    key = (nz["bg"], tuple(nz["b12"]), nz["bfc"])
    if key not in _CACHE:
        _CACHE[key] = _build(nz)
    nc = _CACHE[key]

    def rs_row(gn):   # [ROWS], natural row order
        return np.tile(np.sum(gn, axis=1), BLOC).astype(np.float32)

    def pad_g4(b):
        v = np.zeros(G4, np.float32)
        b = np.asarray(b, np.float32)
        v[:b.shape[0]] = b
        return v

    bds = np.concatenate(
        [_bd_pad_T(g) for g in (gl, g1, g2, gfc)] + [_bd_pad_T(gl) * S8],
        axis=1).astype(F16)
    bvec = np.concatenate([bg * SGATE, pad_g4(b_h1), pad_g4(b_h2),
                           pad_g4(b_fc)])[None, :].astype(np.float32)
    rsv = np.concatenate([rs_row(g) for g in (gl, g1, g2, gfc)])[None, :]

    wh12 = np.concatenate([np.asarray(W_h1, np.float32),
                           np.asarray(W_h2, np.float32)], axis=1)

    # DoubleRowSwInterleave weight packing: per gate tile gt the 256-col
    # block holds stored[:, 2j+i] = W_ktile_i[:, 127-j] (interp semantics:
    # deinterleave then reverse columns restores the logical weights).
    w8 = (w_hh * S8).astype(F8).astype(np.float32)      # quantize first
    w8 = w8.reshape(2, 128, 8, 128)[:, :, :, ::-1]      # [i, p, gt, jrev]
    whh_swi = np.ascontiguousarray(
        w8.transpose(1, 2, 3, 0)).astype(F8)            # [p, gt, j, i]

    shared = {
        "wih": (w_ih * SGATE).astype(F16),
        "whh8": whh_swi.reshape(128, 8, 256),
        "wh12": wh12.astype(F16),
        "wfc": np.ascontiguousarray(
            np.asarray(W_fc, np.float32).reshape(NH, 128, H)
            .transpose(1, 0, 2)).astype(F16),
        "bds": bds,
        "ident": np.eye(128, dtype=F16),
        "bvec": bvec,
        "rsv": np.ascontiguousarray(rsv.astype(np.float32)),
    }

    in_maps = []
    for core in range(NCORES):
        xs = x[core * BLOC:(core + 1) * BLOC]               # [32, T, N, D]
        xr = xs.transpose(1, 0, 2, 3).reshape(T, ROWS, D)   # natural rows
        xch = np.zeros((T, 128, NCHUNK, D), np.float32)
        xv = xr.reshape(T, NCHUNK, CROWS, D)
        xch[:, :CROWS] = xv.transpose(0, 2, 1, 3)           # [t, slot, c, f]
        m = dict(shared)
        m["xc"] = np.ascontiguousarray(xch.astype(F16))
        in_maps.append(m)

    res = run_bass_kernel_spmd(nc, in_maps, core_ids=list(range(NCORES)))
    kernel.last_results = res
    kernel.last_nc = nc
    kernel.last_in_maps = in_maps

    out = np.empty((B, N, H), np.float32)
    yl = np.empty((B, N, H), np.float32)
    for core in range(NCORES):
        r = res.results[core]
        o = r["outT"].reshape(H, ROWS).T.reshape(BLOC, N, H)
        y = r["ylT"].reshape(H, ROWS).T.reshape(BLOC, N, H)
        out[core * BLOC:(core + 1) * BLOC] = o
        yl[core * BLOC:(core + 1) * BLOC] = y
    return out, yl
